# revision 36
# baseline (speedup 1.0000x reference)
"""AttentionWithContext on 8 NeuronCores (Trainium2, Bass/Tile).

Sharding: batch x head-group. Core (b, g) with b in 0..3, g in 0..1 computes
batch b, heads g*8..g*8+8 (Megatron column-parallel QKV, row-parallel proj).
Host pre-transposes/slices weights per core (cast to fp16 host-side), sums
the two partial proj outputs per batch and adds the bias (row-parallel
unshard).

Per-core pipeline (fp16 matmul inputs, fp32 PSUM accumulate):
  qk-proj: qkT[j,n] = wqk[c,j]^T x^T[c,n]      (pair-major j layout)
  v-proj:  v[n,jv] = xT[c,n]^T wvT[c,jv]       (ones column appended)
  QK^T:    S^T[m,n] per head (K=64), kv chunks of 128
  softmax: ACT exp (scale=1/sqrt(d) fused), ones-row of v gives l[n]
  AV:      out^T[d,n] accumulated over kv chunks
  norm:    1/l via DVE fast reciprocal, GpSimd partition-broadcast, DVE mul
  proj:    y[n,co] = OT[ci,n]^T w2T[ci,co], accumulated per pair into SBUF
The output projection and the next pair's q/k projections are emitted as
filler between attention kv-chunks so the PE stays fed while ACT runs exp.

Shapes (fixed): x (4,1024,1024), context (4,256,2048), w_qkv (3072,1024),
w_proj (1024,1024), b_proj (1024,). H=16 heads, D=64, N=1024, N_c=256.
"""
import sys

if "/opt/trn_rl_repo" not in sys.path:
    sys.path.insert(0, "/opt/trn_rl_repo")

from collections import deque

import numpy as np

import concourse.bass as bass
import concourse.mybir as mybir
import concourse.tile as tile
from concourse import bacc, bass_utils

B, N, C = 4, 1024, 1024
H, D = 16, 64
NC_ = 256            # context length
M = NC_ + N          # kv length = 1280
HG = 8               # heads per core
NCORES = 8
SCALE = D ** -0.5

f32 = mybir.dt.float32
DT = mybir.dt.float16
NPDT = np.float16
AF = mybir.ActivationFunctionType

_compiled = None


def _build():
    nc = bacc.Bacc("TRN2", target_bir_lowering=False, debug=False,
                   num_devices=NCORES)
    xT_ap = nc.dram_tensor("xT", [C, N], DT, kind="ExternalInput").ap()
    wqk_ap = nc.dram_tensor("wqkT", [8, C, 128], DT, kind="ExternalInput").ap()
    wv_ap = nc.dram_tensor("wvT", [C, 512], DT, kind="ExternalInput").ap()
    ctxk_ap = nc.dram_tensor("ctxkT", [4, 128, NC_], DT, kind="ExternalInput").ap()
    ctxv_ap = nc.dram_tensor("ctxv", [NC_, 512], DT, kind="ExternalInput").ap()
    w2_ap = nc.dram_tensor("w2T", [512, C], DT, kind="ExternalInput").ap()
    y_ap = nc.dram_tensor("y", [N, C], f32, kind="ExternalOutput").ap()

    CCH = C // 128      # 8 contraction chunks
    MCH = M // 128      # 10 kv chunks
    NCH = N // 128      # 8 query chunks

    with tile.TileContext(nc) as tc:
        with (
            tc.tile_pool(name="sb", bufs=1) as sb,
            tc.tile_pool(name="ps", bufs=1, space="PSUM") as ps,
        ):
            # ---- input loads (fp16 straight from DRAM) ----
            # Spread across engine DMA queues so loads run in parallel;
            # first-needed tensors (x, wqk pair 0) lead each queue.
            xTr = sb.tile([128, CCH, N], DT, tag="r_x")
            xT_src = xT_ap.rearrange("(c p) n -> p c n", p=128)
            wqkr = sb.tile([128, 8, CCH, 128], DT, tag="r_wqk")
            wqk_src = wqk_ap.rearrange("q (c p) j -> p q c j", p=128)
            ctxkTr = sb.tile([128, 4, NC_], DT, tag="r_ctxk")
            ctxvr = sb.tile([128, NC_ // 128, 512], DT, tag="r_ctxv")
            wvTr = sb.tile([128, CCH, 512], DT, tag="r_wv")
            w2Tr = sb.tile([128, 4, C], DT, tag="r_w2")

            qs = [nc.sync, nc.gpsimd, nc.scalar, nc.sync, nc.gpsimd]
            for cc in range(CCH):
                qs[cc % 3].dma_start(xTr[:, cc, :], xT_src[:, cc, :])
            qs[4].dma_start(wqkr[:, 0, :, :], wqk_src[:, 0, :, :])
            qs[4].dma_start(wqkr[:, 4, :, :], wqk_src[:, 4, :, :])
            qs[0].dma_start(ctxkTr[:], ctxk_ap.rearrange("q p m -> p q m"))
            qs[1].dma_start(ctxvr[:],
                            ctxv_ap.rearrange("(c p) j -> p c j", p=128))
            for i, jc in enumerate((1, 5, 2, 6, 3, 7)):
                qs[i % 3].dma_start(wqkr[:, jc, :, :], wqk_src[:, jc, :, :])
            qs[2].dma_start(wvTr[:], wv_ap.rearrange("(c p) j -> p c j", p=128))
            qs[3].dma_start(w2Tr[:], w2_ap.rearrange("(c p) j -> p c j", p=128))

            ones = sb.tile([128, 1], f32, tag="ones")
            nc.gpsimd.memset(ones[:], 1.0)

            v_aug = sb.tile([128, HG, MCH, D + 1], DT, tag="vaug")
            y_acc = sb.tile([128, NCH, C], f32, tag="yacc")
            kT2 = [sb.tile([128, M], DT, tag=f"kT2_{p}", name=f"kT2_{p}")
                   for p in range(4)]
            qT2 = [sb.tile([128, N], DT, tag=f"qT2_{p}", name=f"qT2_{p}")
                   for p in range(4)]
            OT = sb.tile([128, 4, C], DT, tag="OT")

            # v_aug ones column + context k/v (emitted before any attention)
            nc.vector.tensor_copy(
                v_aug[:, :, :, D:D + 1],
                ones[:].to_broadcast((128, HG, MCH, 1)))
            for cc in range(NC_ // 128):
                nc.vector.tensor_copy(
                    v_aug[:, :, cc, 0:D],
                    ctxvr[:, cc, :].rearrange("p (h d) -> p h d", d=D))

            # ---- PE work generators (interleaved into attention) ----
            def emit_qkproj(p):
                """q,k projections for pair p + ctx part of kT2."""
                nc.vector.tensor_copy(kT2[p][:, 0:NC_], ctxkTr[:, p, :])
                for kind, jc in ((0, p), (1, 4 + p)):
                    # cc-outer so both n-halves reuse each loaded stationary
                    qk_ps = [ps.tile([128, 512], f32, tag="b512", bufs=2,
                                     name=f"qk_ps{nh}") for nh in range(2)]
                    for cc in range(CCH):
                        for nh in range(2):
                            nc.tensor.matmul(
                                qk_ps[nh][:],
                                wqkr[:, jc, cc, :],
                                xTr[:, cc, nh * 512:(nh + 1) * 512],
                                start=(cc == 0), stop=(cc == CCH - 1),
                            )
                        if cc % 4 == 3:
                            yield
                    for nh in range(2):
                        if kind == 0:
                            nc.vector.tensor_copy(
                                qT2[p][:, nh * 512:(nh + 1) * 512],
                                qk_ps[nh][:])
                        else:
                            nc.vector.tensor_copy(
                                kT2[p][:, NC_ + nh * 512:NC_ + (nh + 1) * 512],
                                qk_ps[nh][:])

            def emit_vproj():
                """v for all heads; fills v_aug kv-chunks 2..9."""
                for nch in range(NCH):
                    v_ps = ps.tile([128, 512], f32, tag="b512", bufs=2,
                                   name="v_ps")
                    for cc in range(CCH):
                        nc.tensor.matmul(
                            v_ps[:],
                            xTr[:, cc, nch * 128:(nch + 1) * 128],
                            wvTr[:, cc, :],
                            start=(cc == 0), stop=(cc == CCH - 1),
                        )
                    nc.vector.tensor_copy(
                        v_aug[:, :, nch + 2, 0:D],
                        v_ps[:].rearrange("p (h d) -> p h d", d=D))
                    yield

            def emit_projpair(p):
                """Pair p's contribution to y: y_acc (+)= OT[:,p]^T w2T[p]."""
                for nch in range(NCH):
                    for cot in range(2):
                        y_ps = ps.tile([128, 512], f32, tag="b512", bufs=2,
                                       name="y_ps")
                        nc.tensor.matmul(
                            y_ps[:],
                            OT[:, p, nch * 128:(nch + 1) * 128],
                            w2Tr[:, p, cot * 512:(cot + 1) * 512],
                            start=True, stop=True,
                        )
                        dst = y_acc[:, nch, cot * 512:(cot + 1) * 512]
                        if p == 0:
                            nc.vector.tensor_copy(dst, y_ps[:])
                        else:
                            nc.vector.tensor_add(dst, dst, y_ps[:])
                        if p == 3 and cot == 1:
                            qs[nch % 3].dma_start(
                                y_ap[nch * 128:(nch + 1) * 128, :],
                                y_acc[:, nch, :])
                        yield

            def emit_attention(p, filler, pulls=1):
                """Attention for heads 2p, 2p+1; pulls filler work items
                per kv chunk to keep the PE fed while ACT runs exp."""
                for hh in range(2):
                    h = 2 * p + hh
                    hb = hh * 64
                    av_ps = [ps.tile([D + 1, 512], f32, tag="av", bufs=2,
                                     name=f"av{h}_{nt}") for nt in range(2)]
                    # AV runs one kv-chunk behind QK/exp so the next QK is
                    # already in the PE stream when ACT finishes an exp —
                    # otherwise every exp waits out the AV+QK round-trip.
                    eSTs = [None] * MCH
                    for mc in range(MCH):
                        for _ in range(pulls):
                            next(filler)
                        s_ps = ps.tile([128, N], f32, tag="qk1024", bufs=2,
                                       name="s_ps")
                        for nt in range(2):
                            nc.tensor.matmul(
                                s_ps[:, nt * 512:(nt + 1) * 512],
                                kT2[p][hb:hb + 64, mc * 128:(mc + 1) * 128],
                                qT2[p][hb:hb + 64, nt * 512:(nt + 1) * 512],
                                start=True, stop=True,
                            )
                        eST = sb.tile([128, N], DT, tag="eST", bufs=6,
                                      name=f"eST{mc}")
                        nc.scalar.activation(eST[:], s_ps[:], AF.Exp,
                                             scale=float(SCALE))
                        eSTs[mc] = eST
                        if mc > 0:
                            for nt in range(2):
                                nc.tensor.matmul(
                                    av_ps[nt][:],
                                    v_aug[:, h, mc - 1, :],
                                    eSTs[mc - 1][:, nt * 512:(nt + 1) * 512],
                                    start=(mc == 1), stop=False,
                                )
                    for nt in range(2):
                        nc.tensor.matmul(
                            av_ps[nt][:],
                            v_aug[:, h, MCH - 1, :],
                            eSTs[MCH - 1][:, nt * 512:(nt + 1) * 512],
                            start=False, stop=True,
                        )
                    lrow = sb.tile([1, N], f32, tag="lrow", bufs=2,
                                   name=f"lrow{h}")
                    for nt in range(2):
                        nc.vector.tensor_copy(
                            lrow[:, nt * 512:(nt + 1) * 512],
                            av_ps[nt][D:D + 1, :])
                    linv = sb.tile([1, N], f32, tag="linv", bufs=2,
                                   name=f"linv{h}")
                    nc.vector.reciprocal_approx_fast(linv[:], lrow[:])
                    lbc = sb.tile([64, N], f32, tag="lbc", bufs=2,
                                  name=f"lbc{h}")
                    nc.gpsimd.partition_broadcast(lbc[:], linv[:])
                    if p == 3:
                        # normalize straight from PSUM, per n-chunk, so the
                        # final projection pipelines with the last heads
                        for nch in range(NCH):
                            nt, c0 = nch // 4, (nch % 4) * 128
                            nc.vector.tensor_mul(
                                OT[hb:hb + 64, p,
                                   nch * 128:(nch + 1) * 128],
                                av_ps[nt][0:D, c0:c0 + 128],
                                lbc[:, nch * 128:(nch + 1) * 128])
                    else:
                        uo = sb.tile([D, N], f32, tag="uo", bufs=3,
                                     name=f"uo{h}")
                        for nt in range(2):
                            nc.vector.tensor_copy(
                                uo[:, nt * 512:(nt + 1) * 512],
                                av_ps[nt][0:D, :])
                        nc.vector.tensor_mul(OT[hb:hb + 64, p, :], uo[:],
                                             lbc[:])

            # ---- emission schedule ----
            # vproj chunk for kv-chunk mc is always emitted at least one
            # chunk ahead of its first AV use via the per-chunk filler pulls.
            fq = deque([emit_vproj(), emit_qkproj(1)])

            class _Filler:
                def __next__(self):
                    while fq:
                        try:
                            next(fq[0])
                            return
                        except StopIteration:
                            fq.popleft()
            filler = _Filler()

            for _ in emit_qkproj(0):
                pass
            emit_attention(0, filler)
            fq.append(emit_qkproj(2))
            fq.append(emit_projpair(0))
            emit_attention(1, filler)
            fq.append(emit_qkproj(3))
            fq.append(emit_projpair(1))
            emit_attention(2, filler, pulls=2)
            fq.append(emit_projpair(2))
            emit_attention(3, filler, pulls=2)
            fq.append(emit_projpair(3))
            while fq:
                try:
                    next(fq[0])
                except StopIteration:
                    fq.popleft()

    nc.compile()
    return nc


def _get_compiled():
    global _compiled
    if _compiled is None:
        _compiled = _build()
    return _compiled


def _prep_core_inputs(x, context, w_qkv, w_proj):
    """Build the per-core input maps (numpy, host-side sharding + fp16)."""
    in_maps = []
    for core in range(NCORES):
        b, g = core // 2, core % 2
        h0 = g * HG
        xT = np.ascontiguousarray(x[b].T, dtype=NPDT)           # [C, N]
        q_rows = w_qkv[h0 * D:(h0 + HG) * D]                    # [512, C]
        k_rows = w_qkv[C + h0 * D:C + (h0 + HG) * D]
        v_rows = w_qkv[2 * C + h0 * D:2 * C + (h0 + HG) * D]
        # [8 jc-blocks, C, 128]
        wqkT = np.ascontiguousarray(
            np.concatenate([q_rows, k_rows], 0).T.reshape(C, 8, 128)
            .transpose(1, 0, 2), dtype=NPDT)
        wvT = np.ascontiguousarray(v_rows.T, dtype=NPDT)        # [C, 512]
        ctx = context[b].reshape(NC_, 2, H, D)
        ctx_k = ctx[:, 0, h0:h0 + HG, :]                        # [256, 8, 64]
        ctx_v = ctx[:, 1, h0:h0 + HG, :]
        # ctxkT: [4 pairs, 128 = 2 heads x 64 d, 256 m]
        ctxkT = np.ascontiguousarray(
            ctx_k.transpose(1, 2, 0).reshape(4, 128, NC_), dtype=NPDT)
        ctxv = np.ascontiguousarray(ctx_v.reshape(NC_, HG * D), dtype=NPDT)
        w2T = np.ascontiguousarray(w_proj[:, h0 * D:(h0 + HG) * D].T,
                                   dtype=NPDT)                  # [512, C]
        in_maps.append({
            "xT": xT, "wqkT": wqkT, "wvT": wvT,
            "ctxkT": ctxkT, "ctxv": ctxv, "w2T": w2T,
        })
    return in_maps


def kernel(x, context, w_qkv, w_proj, b_proj, _trace=False):
    x = np.asarray(x, dtype=np.float32)
    context = np.asarray(context, dtype=np.float32)
    w_qkv = np.asarray(w_qkv, dtype=np.float32)
    w_proj = np.asarray(w_proj, dtype=np.float32)
    b_proj = np.asarray(b_proj, dtype=np.float32)

    nc = _get_compiled()
    in_maps = _prep_core_inputs(x, context, w_qkv, w_proj)
    res = bass_utils.run_bass_kernel_spmd(
        nc, in_maps, list(range(NCORES)), trace=_trace)
    kernel.last_results = res

    out = np.empty((B, N, C), np.float32)
    for b in range(B):
        out[b] = (res.results[2 * b]["y"] + res.results[2 * b + 1]["y"]
                  + b_proj)
    return out
